# revision 33
# baseline (speedup 1.0000x reference)
"""Additive (Bahdanau) attention on 8 TRN2 NeuronCores.

Reference computation (per batch element b, one NeuronCore each):
    q  = queries @ W_q.T                      # (Q, H)
    k  = keys @ W_k.T                         # (K, H)
    s[q,k] = sum_h w_v[h] * tanh(q[q,h] + k[k,h])
    s[q,k] += mask (0 valid / -big invalid)
    attn = softmax_k(s)
    out  = attn @ values                      # (Q, Dv)

Shapes: B=8, Q=128, K=1024, D=256, H=256 (hardcoded; data-parallel over B).

Key idea: tanh(z) ~= sum_r a_r sin(w_r z), and sin(w(q+k)) =
sin(wq)cos(wk) + cos(wq)sin(wk) is SEPARABLE: the (Q,K,H) elementwise
tanh (33.5M ScalarE elements, 219us roofline of the direct approach)
collapses into TensorE matmuls over per-frequency sin/cos FEATURES of
the two projections, with contraction dim H * n_features.

Frequency lattice (8 effective freqs from 5 bases, balancing engines):
  * bases w0..w4: k-side features via DVE reflection-fold range
    reduction + one ScalarE Sin pass each.  cos is even & 2pi-periodic,
    so cos(t)=cos(||t|-4pi|)=...; each fold is one fused tensor_scalar
    (abs, subtract) pass at 4x fp16.  (The DVE ISA has no abs in bass's
    enum, but walrus accepts op="abs": we emit max-placeholders and
    patch the serialized BIR - see _patch_bir.)
  * doubled freqs 2*w2..2*w4: NO new transcendentals - sin(2a)=2 s c is
    one DVE tensor_tensor product, cos(2a)=1-2s^2 comes from s^2 via
    the ScalarE Square activation, with the affine constants absorbed
    by a shared ones-feature chunk and the host-built G scaling.
  * mask folded in as a rank-1 matmul accumulate (start=True)
  * fixed-shift softmax (exp(s-5) with ScalarE accum_out row sums),
    PE transposes, attn @ V.
All score matmuls are emitted after feature production so the PE p-state
ramp (1.2 -> 2.4 GHz after ~3us of continuous work) stays warm.
"""

import numpy as np

B, Q, K, D, H = 8, 128, 1024, 256, 256
NEG_BIG = -60000.0

WB = [0.2539, 0.75966, 1.22824, 1.51757, 1.92331]
NEFF = 8                      # effective freqs: WB + 2*WB[2:]
# lstsq coefficients in deployed-feature space (see transcript emu3.py):
# per base i: (G=sin x F_cos, G=cos x F_sin); per double j: (G=cos x Ps,
# G=sin x Pc, G=sin x ones)
C_BASE = [(1.2398964, 1.23993646), (0.33426951, 0.33430961),
          (0.11794401, 0.11793227), (0.05075872, 0.05078806),
          (0.04918963, 0.0491699)]
C_DBL = [(0.04662311, -0.04664259, 0.02330235),
         (0.02157817, -0.02159513, 0.01081808),
         (0.01002109, -0.01004415, 0.00502048)]
XMAX = 5.5
PI = float(np.pi)


def _nfolds(w):
    u0max = w * XMAX + PI / 2
    if u0max <= PI - 0.05:
        return 0
    if u0max <= 2 * PI - 0.1:
        return 1
    if u0max <= 4 * PI - 0.1:
        return 2
    return 3


_CACHE = {}


def _build_bass():
    import concourse.bass as bass
    import concourse.tile as tile
    from concourse import mybir
    from concourse.masks import make_identity
    from contextlib import ExitStack

    F32 = mybir.dt.float32
    F16 = mybir.dt.float16
    AF = mybir.ActivationFunctionType
    OP = mybir.AluOpType

    nc = bass.Bass()
    abs_patch = []

    qT_ext = nc.declare_dram_parameter("qT", [D, Q], F16, isOutput=False)
    kT_ext = nc.declare_dram_parameter("kT", [D, K], F16, isOutput=False)
    vals_ext = nc.declare_dram_parameter("vals", [K, D], F16, isOutput=False)
    wqT_ext = nc.declare_dram_parameter("wqT", [D, H], F16, isOutput=False)
    wkT_ext = nc.declare_dram_parameter("wkT", [D, H], F16, isOutput=False)
    m_ext = nc.declare_dram_parameter("M", [128, NEFF, 2, 2, Q], F16, isOutput=False)
    mw_ext = nc.declare_dram_parameter("Mw", [128, 2, Q], F16, isOutput=False)
    mask_ext = nc.declare_dram_parameter("mask", [1, K], F16, isOutput=False)
    out_ext = nc.declare_dram_parameter("out", [Q, D], F32, isOutput=True)

    with tile.TileContext(nc) as tc, ExitStack() as ctx:
        persist = ctx.enter_context(tc.tile_pool(name="persist", bufs=1))
        scores_ps = ctx.enter_context(tc.tile_pool(name="scores_ps", bufs=1, space="PSUM"))
        argk_pool = ctx.enter_context(tc.tile_pool(name="argk_pool", bufs=2))
        f_pool = ctx.enter_context(tc.tile_pool(name="f_pool", bufs=5))
        p_pool = ctx.enter_context(tc.tile_pool(name="p_pool", bufs=3))

        # ---- persistent SBUF tiles ----
        qT_sb = persist.tile([128, 2, Q], F16)
        kT_sb = persist.tile([128, 2, K], F16)
        wqT_sb = persist.tile([128, 2, H], F16)
        wkT_sb = persist.tile([128, 2, H], F16)
        val_sb = persist.tile([128, 8, D], F16)
        m_sb = persist.tile([128, NEFF, 2, 2, Q], F16)
        mw_sb = persist.tile([128, 2, Q], F16)          # w_v broadcast
        mask_sb = persist.tile([1, K], F16)
        ones_sb = persist.tile([1, 128], F16)
        onesk_sb = persist.tile([128, 512], F16)        # ones-feature moving
        ident = persist.tile([128, 128], F16)
        qf_sb = persist.tile([128, 2, Q], F16)
        kf_sb = persist.tile([128, 2, K], F16)
        zk_sb = persist.tile([128, 2, K], F16)          # |kf|
        zq_sb = persist.tile([128, 2, Q], F32)          # |qf|
        argq_a = persist.tile([128, NEFF, 2, 2, Q], F32)
        argq_b = persist.tile([128, NEFF, 2, 2, Q], F32)
        argq_f = persist.tile([128, NEFF, 2, 2, Q], F32)
        sincos_q = persist.tile([128, NEFF, 2, 2, Q], F16)
        g_sb = persist.tile([128, NEFF, 2, 2, Q], F16)
        gone_sb = persist.tile([128, 2, Q], F16)        # ones-chunk stationary
        u1_sb = persist.tile([128, 2, Q], F16)
        u2_sb = persist.tile([128, 2, Q], F16)
        u3_sb = persist.tile([128, 2, Q], F16)
        E_q0 = persist.tile([128, K // 4], F16)
        E_q1 = persist.tile([128, K // 4], F16)
        E_q2 = persist.tile([128, K // 4], F16)
        E_q3 = persist.tile([128, K // 4], F16)
        E_q = [E_q0, E_q1, E_q2, E_q3]
        ET_sb = persist.tile([128, 8, 128], F16)
        out_sb = persist.tile([Q, D], F32)
        pihalf = persist.tile([128, 1], F32)
        zero_b = persist.tile([128, 1], F32)
        shift_sb = persist.tile([128, 1], F32)
        dummy = persist.tile([128, 1], F32)
        rs_q0 = persist.tile([128, 1], F32)
        rs_q1 = persist.tile([128, 1], F32)
        rs_q2 = persist.tile([128, 1], F32)
        rs_q3 = persist.tile([128, 1], F32)
        rs_q = [rs_q0, rs_q1, rs_q2, rs_q3]
        rowsum = persist.tile([128, 1], F32)
        rs_t = persist.tile([128, 1], F32)
        rinv = persist.tile([128, 1], F32)

        # ---- DMA inputs (kT chain first: it gates the ScalarE stream) ----
        nc.sync.dma_start(out=kT_sb[:, 0, 0:512], in_=kT_ext[0:128, 0:512])
        nc.sync.dma_start(out=wkT_sb, in_=wkT_ext.rearrange("(t p) h -> p t h", p=128))
        nc.sync.dma_start(out=kT_sb[:, 1, 0:512], in_=kT_ext[128:256, 0:512])
        nc.sync.dma_start(out=kT_sb[:, 0, 512:1024], in_=kT_ext[0:128, 512:1024])
        nc.sync.dma_start(out=kT_sb[:, 1, 512:1024], in_=kT_ext[128:256, 512:1024])
        nc.sync.dma_start(out=qT_sb, in_=qT_ext.rearrange("(t p) q -> p t q", p=128))
        nc.sync.dma_start(out=wqT_sb, in_=wqT_ext.rearrange("(t p) h -> p t h", p=128))
        nc.sync.dma_start(out=m_sb, in_=m_ext[:, :, :, :, :])
        nc.sync.dma_start(out=mw_sb, in_=mw_ext[:, :, :])
        nc.sync.dma_start(out=mask_sb, in_=mask_ext[:, :])
        nc.sync.dma_start(out=val_sb, in_=vals_ext.rearrange("(t p) v -> p t v", p=128))
        nc.vector.memset(ones_sb, 1.0)
        nc.vector.memset(onesk_sb, 1.0)
        nc.vector.memset(pihalf, PI / 2)
        nc.vector.memset(zero_b, 0.0)
        nc.vector.memset(shift_sb, -5.0)
        make_identity(nc, ident)
        # trigger the Sin table load at t~0 (hidden under DMA/projections)
        nc.scalar.activation(dummy, pihalf, AF.Sin, scale=0.1)

        def ts_abs(out, in_, s1, s2, op0, op1, patch):
            if op1 is None:
                i = nc.vector.tensor_scalar(out, in_, s1, s2, op0=op0)
            else:
                i = nc.vector.tensor_scalar(out, in_, s1, s2, op0=op0, op1=op1)
            abs_patch.append((i.ins.name, patch))
            return i

        scores_a = scores_ps.tile([128, K // 2], F32, tag="sca")
        scores_b = scores_ps.tile([128, K // 2], F32, tag="scb")
        scores_c = [scores_a, scores_b]

        setup_ctx = ExitStack()
        qf_ps_pool = setup_ctx.enter_context(
            tc.tile_pool(name="qf_ps", bufs=1, space="PSUM"))
        kf_ps_pool = setup_ctx.enter_context(
            tc.tile_pool(name="kf_ps", bufs=1, space="PSUM"))

        # ---- projections ----
        ps_q0 = qf_ps_pool.tile([128, Q], F32, tag="psq0")
        ps_q1 = qf_ps_pool.tile([128, Q], F32, tag="psq1")
        for ht, psq in ((0, ps_q0), (1, ps_q1)):
            hsl = slice(ht * 128, (ht + 1) * 128)
            nc.tensor.matmul(psq, wqT_sb[:, 0, hsl], qT_sb[:, 0, :],
                             start=True, stop=False)
            nc.tensor.matmul(psq, wqT_sb[:, 1, hsl], qT_sb[:, 1, :],
                             start=False, stop=True)
            nc.scalar.copy(qf_sb[:, ht, :], psq)

        kf0 = kf_ps_pool.tile([128, K], F32, tag="kf0")
        kf1 = kf_ps_pool.tile([128, K], F32, tag="kf1")
        for ht, kfp in ((0, kf0), (1, kf1)):
            hsl = slice(ht * 128, (ht + 1) * 128)
            for c in range(2):
                csl = slice(c * 512, (c + 1) * 512)
                nc.tensor.matmul(kfp[:, csl], wkT_sb[:, 0, hsl],
                                 kT_sb[:, 0, csl], start=True, stop=False)
                nc.tensor.matmul(kfp[:, csl], wkT_sb[:, 1, hsl],
                                 kT_sb[:, 1, csl], start=False, stop=True)
            nc.scalar.copy(kf_sb[:, ht, :], kfp)
        setup_ctx.close()

        # mask rank-1 (start=True) - first writers of the score banks
        for c in range(2):
            csl = slice(c * 512, (c + 1) * 512)
            nc.tensor.matmul(scores_c[c], ones_sb, mask_sb[:, csl],
                             start=True, stop=False)

        ts_abs(zk_sb, kf_sb, 0.0, None, op0=OP.max, op1=None, patch="op0")

        def emit_kfeat(i):
            w = WB[i]
            nf = _nfolds(w)
            F = f_pool.tile([128, 2, 2, K], F16, tag="F")
            if nf == 0:
                nc.scalar.activation(F[:, 0, :, :], kf_sb, AF.Sin,
                                     bias=zero_b, scale=w)
                nc.scalar.activation(F[:, 1, :, :], kf_sb, AF.Sin,
                                     bias=pihalf, scale=w)
                return F
            C = [4 * PI, 2 * PI, PI][3 - nf:]
            s_w = PI / (2 * w)
            a1 = argk_pool.tile([128, 2, 2, K], F16, tag="ka")
            a2 = argk_pool.tile([128, 2, 2, K], F16, tag="kb")
            ts_abs(a1[:, 0, :, :], kf_sb, s_w, 0.0,
                   op0=OP.subtract, op1=OP.max, patch="op1")
            nc.vector.tensor_scalar(a2[:, 0, :, :], a1[:, 0, :, :], w, C[0],
                                    op0=OP.mult, op1=OP.subtract)
            nc.vector.tensor_scalar(a2[:, 1, :, :], zk_sb, w, C[0],
                                    op0=OP.mult, op1=OP.subtract)
            cur, other = a2, a1
            for c in list(C[1:]) + [PI / 2]:
                ts_abs(other, cur, 0.0, c, op0=OP.max, op1=OP.subtract,
                       patch="op0")
                cur, other = other, cur
            nc.scalar.activation(F, cur, AF.Sin, bias=zero_b, scale=1.0)
            return F

        def emit_qside_r0():
            # j=0 is fold-free: emit its acts + G right after the qf copies
            # so the first score matmuls (chunk j0) release the PE early
            w = WB[0]
            nc.scalar.activation(sincos_q[:, 0, 0, :, :], qf_sb,
                                 AF.Sin, bias=zero_b, scale=w)
            nc.scalar.activation(sincos_q[:, 0, 1, :, :], qf_sb,
                                 AF.Sin, bias=pihalf, scale=w)
            nc.vector.tensor_mul(g_sb[:, 0, :, :, :],
                                 sincos_q[:, 0, :, :, :],
                                 m_sb[:, 0, :, :, :])

        def emit_qside():
            W_EFF = WB + [2 * w for w in WB[2:]]
            ts_abs(zq_sb, qf_sb, 0.0, None, op0=OP.max, op1=None, patch="op0")
            for j, w in enumerate(W_EFF):
                nf = _nfolds(w)
                if nf == 0:
                    nc.scalar.activation(sincos_q[:, j, 0, :, :], qf_sb,
                                         AF.Sin, bias=zero_b, scale=w)
                    nc.scalar.activation(sincos_q[:, j, 1, :, :], qf_sb,
                                         AF.Sin, bias=pihalf, scale=w)
                    continue
                C = [4 * PI, 2 * PI, PI][3 - nf:]
                s_w = PI / (2 * w)
                ts_abs(argq_a[:, j, 0, :, :], qf_sb, s_w, 0.0,
                       op0=OP.subtract, op1=OP.max, patch="op1")
                nc.vector.tensor_scalar(argq_b[:, j, 0, :, :],
                                        argq_a[:, j, 0, :, :], w, C[0],
                                        op0=OP.mult, op1=OP.subtract)
                nc.vector.tensor_scalar(argq_b[:, j, 1, :, :], zq_sb, w, C[0],
                                        op0=OP.mult, op1=OP.subtract)
                L = list(C[1:]) + [PI / 2]
                seq = [argq_b, argq_a, argq_b]
                for ii, c in enumerate(L):
                    dst = argq_f if ii == len(L) - 1 else seq[ii + 1]
                    ts_abs(dst[:, j, :, :, :], seq[ii][:, j, :, :, :], 0.0, c,
                           op0=OP.max, op1=OP.subtract, patch="op0")
            nc.scalar.activation(sincos_q[:, 1:NEFF, :, :, :],
                                 argq_f[:, 1:NEFF, :, :, :],
                                 AF.Sin, bias=zero_b, scale=1.0)
            # G stationaries: per-j TTs (j=0 depends only on the early
            # r0 acts, releasing the first score matmuls ~10us sooner than
            # one full-tile TT would).  Double chunks pair the sin-G with
            # Pc and the cos-G with Ps (F side flipped in the chunk table)
            # so no swapped sincos reads are needed.
            for j in range(NEFF):
                nc.vector.tensor_mul(g_sb[:, j, :, :, :],
                                     sincos_q[:, j, :, :, :],
                                     m_sb[:, j, :, :, :])
            # ones-chunk stationary: w_v * sum_j c_j sin(2 w_j q)
            nc.vector.tensor_scalar_mul(u1_sb, sincos_q[:, 5, 0, :, :],
                                        float(C_DBL[0][2]))
            nc.vector.scalar_tensor_tensor(u2_sb, sincos_q[:, 6, 0, :, :],
                                           float(C_DBL[1][2]), u1_sb,
                                           op0=OP.mult, op1=OP.add)
            nc.vector.scalar_tensor_tensor(u3_sb, sincos_q[:, 7, 0, :, :],
                                           float(C_DBL[2][2]), u2_sb,
                                           op0=OP.mult, op1=OP.add)
            nc.vector.tensor_mul(gone_sb, u3_sb, mw_sb)

        # ---- feature production: ALL k-side fold chains first so the
        # in-order DVE queue never makes a k-act wait behind the 22 q-side
        # chain instructions (was a 4+4us ScalarE stall mid-stream) ----
        F_t = {}
        for i in range(5):
            F_t[i] = emit_kfeat(i)
        emit_qside()
        P_t = {}
        for idx, i in enumerate((2, 3, 4)):
            P = p_pool.tile([128, 2, 2, K], F16, tag="P")
            # Ps = s*c (DVE);  Pc = s^2 (ScalarE Square)
            nc.vector.tensor_mul(P[:, 0, :, :], F_t[i][:, 0, :, :],
                                 F_t[i][:, 1, :, :])
            nc.scalar.activation(P[:, 1, :, :], F_t[i][:, 0, :, :], AF.Square,
                                 bias=zero_b, scale=1.0)
            P_t[5 + idx] = P

        # ---- score matmuls, all deferred (keeps the PE p-state ramped) ----
        # chunk list: (stationary slice provider, moving tile, moving side)
        chunks = []
        for i in range(5):
            chunks.append((("g", i, 0), F_t[i], 1))
            chunks.append((("g", i, 1), F_t[i], 0))
        for j in (5, 6, 7):
            # g[:,j,0] = M0*sin(2w q) pairs Pc (side 1); g[:,j,1] = M1*cos
            # pairs Ps (side 0)
            chunks.append((("g", j, 0), P_t[j], 1))
            chunks.append((("g", j, 1), P_t[j], 0))

        def stat(spec, t):
            kind, j, gs = spec
            return g_sb[:, j, gs, t, :]

        for spec, Fm, fs in chunks:
            for t in range(2):
                for c in range(2):
                    csl = slice(c * 512, (c + 1) * 512)
                    nc.tensor.matmul(scores_c[c], stat(spec, t),
                                     Fm[:, fs, t, csl], start=False, stop=False)
        # ones chunk last: bank-A pair first, with per-bank stop flags
        for c in range(2):
            for t in range(2):
                nc.tensor.matmul(scores_c[c], gone_sb[:, t, :], onesk_sb,
                                 start=False, stop=(t == 1))

        # ---- masked softmax: exp(s-5) with accumulated row sums ----
        for qtr in range(4):
            sc = scores_c[qtr // 2]
            off = (qtr % 2) * 256
            nc.scalar.activation(E_q[qtr], sc[:, off:off + 256], AF.Exp,
                                 bias=shift_sb, scale=1.0, accum_out=rs_q[qtr])
        nc.vector.tensor_add(rowsum, rs_q[0], rs_q[1])
        nc.vector.tensor_add(rs_t, rs_q[2], rs_q[3])
        nc.vector.tensor_add(rowsum, rowsum, rs_t)
        nc.vector.reciprocal(rinv, rowsum)

        # ---- attn @ values ----
        with ExitStack() as tail_ctx:
            tp_ps = tail_ctx.enter_context(
                tc.tile_pool(name="tp_ps", bufs=2, space="PSUM"))
            av_ps = tail_ctx.enter_context(
                tc.tile_pool(name="av_ps", bufs=1, space="PSUM"))
            for kt in range(8):
                E_src = E_q[kt // 2]
                off = (kt % 2) * 128
                tp = tp_ps.tile([128, 128], F16, tag="tp")
                nc.tensor.transpose(tp, E_src[:, off:off + 128], ident)
                nc.vector.tensor_copy(ET_sb[:, kt, :], tp)
            ps_av = av_ps.tile([Q, D], F32)
            for kt in range(8):
                nc.tensor.matmul(ps_av, ET_sb[:, kt, :], val_sb[:, kt, :],
                                 start=(kt == 0), stop=(kt == 7))
            nc.vector.tensor_scalar_mul(out_sb[:, 0:128], ps_av[:, 0:128], rinv)
            nc.sync.dma_start(out=out_ext[:, 0:128], in_=out_sb[:, 0:128])
            nc.vector.tensor_scalar_mul(out_sb[:, 128:256], ps_av[:, 128:256], rinv)
        nc.sync.dma_start(out=out_ext[:, 128:256], in_=out_sb[:, 128:256])

    _patch_bir(nc, abs_patch)
    return nc


def _patch_bir(nc, abs_patch):
    """Rewrite recorded max-placeholders to the walrus "abs" ALU op, and
    fix multi-wait instructions (walrus accepts one sync wait each): drop
    redundant same-engine waits on in-order compute queues, hoist the rest
    onto single-wait EventSemaphore carriers."""
    import json

    d = json.loads(nc.to_json_bytes())
    patch_map = dict(abs_patch)
    k = [0]
    self_drop = {"Activation": "Activation", "DVE": "DVE"}
    compute_ops = {"Activation", "TensorScalarPtr", "TensorScalar", "TensorTensor",
                   "TensorCopy", "TensorReduce", "Reciprocal", "Memset"}
    n_abs = 0
    for fn in d["functions"]:
        for blk in fn["blocks"]:
            out = []
            for inst in blk["instructions"]:
                slot = patch_map.get(inst.get("name"))
                if slot is not None:
                    inst[slot] = "abs"
                    n_abs += 1
                si = inst.get("sync_info") or {}
                ow = si.get("on_wait") or []
                op = inst.get("opcode")
                eng = inst.get("engine")
                if len(ow) > 1 and op != "EventSemaphore":
                    if op in compute_ops and eng in self_drop:
                        pref = self_drop[eng] + "_"
                        ow = [w for w in ow
                              if not str(w.get("ant_name", "")).startswith(pref)]
                    while len(ow) > 1:
                        w = ow.pop(0)
                        k[0] += 1
                        out.append({
                            "debug": inst.get("debug", 0), "engine": eng,
                            "ins": [], "name": f"WSplit-{k[0]}",
                            "opcode": "EventSemaphore", "outs": [],
                            "sync_info": {"on_update": [], "on_wait": [w]},
                        })
                    si["on_wait"] = ow
                out.append(inst)
            blk["instructions"] = out
    assert n_abs == len(abs_patch), (n_abs, len(abs_patch))
    patched = json.dumps(d).encode()
    nc.to_json_bytes = lambda: patched


def _get_nc():
    if "nc" not in _CACHE:
        _CACHE["nc"] = _build_bass()
    return _CACHE["nc"]


def _host_prep(queries, keys, values, W_q, W_k, w_v, valid_lens):
    """Build the 8 per-core input maps."""
    queries = np.asarray(queries, dtype=np.float32)
    keys = np.asarray(keys, dtype=np.float32)
    values = np.asarray(values, dtype=np.float32)
    W_q = np.asarray(W_q, dtype=np.float32)
    W_k = np.asarray(W_k, dtype=np.float32)
    w_v = np.asarray(w_v, dtype=np.float32)
    valid = np.asarray(valid_lens).astype(np.int64)

    wqT = np.ascontiguousarray(W_q.T.astype(np.float16))
    wkT = np.ascontiguousarray(W_k.T.astype(np.float16))
    # M[p, j, side, t, q] = w_v[t*128 + p] * coef[j, side]
    wv2 = w_v.reshape(2, 128).T                              # (p, t)
    coef = np.zeros((NEFF, 2), np.float32)
    for i, (c0, c1) in enumerate(C_BASE):
        coef[i] = (c0, c1)
    for idx, (cps, cpc, _) in enumerate(C_DBL):
        coef[5 + idx] = (cpc, cps)      # side0 = sin-G (pairs Pc), side1 = cos-G
    M = np.ascontiguousarray(
        (wv2[:, None, None, :, None] * coef[None, :, :, None, None]
         ).astype(np.float16) * np.ones((1, 1, 1, 1, Q), np.float16))
    Mw = np.ascontiguousarray(
        np.broadcast_to(wv2[:, :, None], (128, 2, Q)).astype(np.float16))

    kidx = np.arange(K)
    in_maps = []
    for b in range(B):
        mask = np.where(kidx < valid[b], np.float16(0.0), np.float16(NEG_BIG))
        in_maps.append({
            "qT": np.ascontiguousarray(queries[b].T.astype(np.float16)),
            "kT": np.ascontiguousarray(keys[b].T.astype(np.float16)),
            "vals": np.ascontiguousarray(values[b].astype(np.float16)),
            "wqT": wqT,
            "wkT": wkT,
            "M": M,
            "Mw": Mw,
            "mask": np.ascontiguousarray(mask.reshape(1, K)),
        })
    return in_maps, valid, values


def _run(inputs, trace=False, **kw):
    from concourse.bass_utils import run_bass_kernel_spmd

    nc = _get_nc()
    in_maps, valid, values = _host_prep(**inputs)
    res = run_bass_kernel_spmd(nc, in_maps, list(range(B)), trace=trace, **kw)
    out = np.stack([np.asarray(res.results[i]["out"], dtype=np.float32)
                    for i in range(B)])
    for b in range(B):
        if valid[b] == 0:
            out[b] = np.broadcast_to(values[b].mean(axis=0), (Q, D))
    return out, res


def kernel(**inputs):
    out, _ = _run(inputs, trace=False)
    return out


# revision 34
# speedup vs baseline: 1.0742x; 1.0742x over previous
"""Additive (Bahdanau) attention on 8 TRN2 NeuronCores.

Reference computation (per batch element b, one NeuronCore each):
    q  = queries @ W_q.T                      # (Q, H)
    k  = keys @ W_k.T                         # (K, H)
    s[q,k] = sum_h w_v[h] * tanh(q[q,h] + k[k,h])
    s[q,k] += mask (0 valid / -big invalid)
    attn = softmax_k(s)
    out  = attn @ values                      # (Q, Dv)

Shapes: B=8, Q=128, K=1024, D=256, H=256 (hardcoded; data-parallel over B).

Key idea: tanh(z) ~= sum_r a_r sin(w_r z), and sin(w(q+k)) =
sin(wq)cos(wk) + cos(wq)sin(wk) is SEPARABLE: the (Q,K,H) elementwise
tanh (33.5M ScalarE elements, 219us roofline of the direct approach)
collapses into TensorE matmuls over per-frequency sin/cos FEATURES of
the two projections, with contraction dim H * n_features.

Frequency lattice (8 effective freqs from 5 bases, balancing engines):
  * bases w0..w4: k-side features via DVE reflection-fold range
    reduction + one ScalarE Sin pass each.  cos is even & 2pi-periodic,
    so cos(t)=cos(||t|-4pi|)=...; each fold is one fused tensor_scalar
    (abs, subtract) pass at 4x fp16.  (The DVE ISA has no abs in bass's
    enum, but walrus accepts op="abs": we emit max-placeholders and
    patch the serialized BIR - see _patch_bir.)
  * doubled freqs 2*w2..2*w4: NO new transcendentals - sin(2a)=2 s c is
    one DVE tensor_tensor product, cos(2a)=1-2s^2 comes from s^2 via
    the ScalarE Square activation, with the affine constants absorbed
    by a shared ones-feature chunk and the host-built G scaling.
  * mask folded in as a rank-1 matmul accumulate (start=True)
  * fixed-shift softmax (exp(s-5) with ScalarE accum_out row sums),
    PE transposes, attn @ V.
All score matmuls are emitted after feature production so the PE p-state
ramp (1.2 -> 2.4 GHz after ~3us of continuous work) stays warm.
"""

import numpy as np

B, Q, K, D, H = 8, 128, 1024, 256, 256
NEG_BIG = -60000.0

WB = [0.2539, 0.75966, 1.22824, 1.51757, 1.92331]
NEFF = 8                      # effective freqs: WB + 2*WB[2:]
# lstsq coefficients in deployed-feature space (see transcript emu3.py):
# per base i: (G=sin x F_cos, G=cos x F_sin); per double j: (G=cos x Ps,
# G=sin x Pc, G=sin x ones)
C_BASE = [(1.2398964, 1.23993646), (0.33426951, 0.33430961),
          (0.11794401, 0.11793227), (0.05075872, 0.05078806),
          (0.04918963, 0.0491699)]
C_DBL = [(0.04662311, -0.04664259, 0.02330235),
         (0.02157817, -0.02159513, 0.01081808),
         (0.01002109, -0.01004415, 0.00502048)]
XMAX = 5.5
PI = float(np.pi)


def _nfolds(w):
    u0max = w * XMAX + PI / 2
    if u0max <= PI - 0.05:
        return 0
    if u0max <= 2 * PI - 0.1:
        return 1
    if u0max <= 4 * PI - 0.1:
        return 2
    return 3


_CACHE = {}


def _build_bass():
    import concourse.bass as bass
    import concourse.tile as tile
    from concourse import mybir
    from concourse.masks import make_identity
    from contextlib import ExitStack

    F32 = mybir.dt.float32
    F16 = mybir.dt.float16
    AF = mybir.ActivationFunctionType
    OP = mybir.AluOpType

    nc = bass.Bass()
    abs_patch = []

    qT_ext = nc.declare_dram_parameter("qT", [D, Q], F16, isOutput=False)
    kT_ext = nc.declare_dram_parameter("kT", [D, K], F16, isOutput=False)
    vals_ext = nc.declare_dram_parameter("vals", [K, D], F16, isOutput=False)
    wqT_ext = nc.declare_dram_parameter("wqT", [D, H], F16, isOutput=False)
    wkT_ext = nc.declare_dram_parameter("wkT", [D, H], F16, isOutput=False)
    m_ext = nc.declare_dram_parameter("M", [128, NEFF, 2, 2, Q], F16, isOutput=False)
    mw_ext = nc.declare_dram_parameter("Mw", [128, 2, Q], F16, isOutput=False)
    mask_ext = nc.declare_dram_parameter("mask", [1, K], F16, isOutput=False)
    out_ext = nc.declare_dram_parameter("out", [Q, D], F32, isOutput=True)

    with tile.TileContext(nc) as tc, ExitStack() as ctx:
        persist = ctx.enter_context(tc.tile_pool(name="persist", bufs=1))
        scores_ps = ctx.enter_context(tc.tile_pool(name="scores_ps", bufs=1, space="PSUM"))
        argk_pool = ctx.enter_context(tc.tile_pool(name="argk_pool", bufs=2))
        f_pool = ctx.enter_context(tc.tile_pool(name="f_pool", bufs=5))
        p_pool = ctx.enter_context(tc.tile_pool(name="p_pool", bufs=3))

        # ---- persistent SBUF tiles ----
        qT_sb = persist.tile([128, 2, Q], F16)
        kT_sb = persist.tile([128, 2, K], F16)
        wqT_sb = persist.tile([128, 2, H], F16)
        wkT_sb = persist.tile([128, 2, H], F16)
        val_sb = persist.tile([128, 8, D], F16)
        m_sb = persist.tile([128, NEFF, 2, 2, Q], F16)
        mw_sb = persist.tile([128, 2, Q], F16)          # w_v broadcast
        mask_sb = persist.tile([1, K], F16)
        ones_sb = persist.tile([1, 128], F16)
        onesk_sb = persist.tile([128, 512], F16)        # ones-feature moving
        ident = persist.tile([128, 128], F16)
        qf_sb = persist.tile([128, 2, Q], F16)
        kf_sb = persist.tile([128, 2, K], F16)
        zk_sb = persist.tile([128, 2, K], F16)          # |kf|
        zq_sb = persist.tile([128, 2, Q], F32)          # |qf|
        argq_a = persist.tile([128, NEFF, 2, 2, Q], F32)
        argq_b = persist.tile([128, NEFF, 2, 2, Q], F32)
        argq_f = persist.tile([128, NEFF, 2, 2, Q], F32)
        sincos_q = persist.tile([128, NEFF, 2, 2, Q], F16)
        g_sb = persist.tile([128, NEFF, 2, 2, Q], F16)
        gone_sb = persist.tile([128, 2, Q], F16)        # ones-chunk stationary
        u1_sb = persist.tile([128, 2, Q], F16)
        u2_sb = persist.tile([128, 2, Q], F16)
        u3_sb = persist.tile([128, 2, Q], F16)
        E_q0 = persist.tile([128, K // 4], F16)
        E_q1 = persist.tile([128, K // 4], F16)
        E_q2 = persist.tile([128, K // 4], F16)
        E_q3 = persist.tile([128, K // 4], F16)
        E_q = [E_q0, E_q1, E_q2, E_q3]
        ET_sb = persist.tile([128, 8, 128], F16)
        out_sb = persist.tile([Q, D], F32)
        pihalf = persist.tile([128, 1], F32)
        zero_b = persist.tile([128, 1], F32)
        shift_sb = persist.tile([128, 1], F32)
        dummy = persist.tile([128, 1], F32)
        rs_q0 = persist.tile([128, 1], F32)
        rs_q1 = persist.tile([128, 1], F32)
        rs_q2 = persist.tile([128, 1], F32)
        rs_q3 = persist.tile([128, 1], F32)
        rs_q = [rs_q0, rs_q1, rs_q2, rs_q3]
        rowsum = persist.tile([128, 1], F32)
        rs_t = persist.tile([128, 1], F32)
        rinv = persist.tile([128, 1], F32)

        # ---- DMA inputs (kT chain first: it gates the ScalarE stream) ----
        nc.sync.dma_start(out=kT_sb[:, 0, 0:512], in_=kT_ext[0:128, 0:512])
        nc.sync.dma_start(out=wkT_sb, in_=wkT_ext.rearrange("(t p) h -> p t h", p=128))
        nc.sync.dma_start(out=kT_sb[:, 1, 0:512], in_=kT_ext[128:256, 0:512])
        nc.sync.dma_start(out=kT_sb[:, 0, 512:1024], in_=kT_ext[0:128, 512:1024])
        nc.sync.dma_start(out=kT_sb[:, 1, 512:1024], in_=kT_ext[128:256, 512:1024])
        nc.sync.dma_start(out=qT_sb, in_=qT_ext.rearrange("(t p) q -> p t q", p=128))
        nc.sync.dma_start(out=wqT_sb, in_=wqT_ext.rearrange("(t p) h -> p t h", p=128))
        nc.sync.dma_start(out=m_sb, in_=m_ext[:, :, :, :, :])
        nc.sync.dma_start(out=mw_sb, in_=mw_ext[:, :, :])
        nc.sync.dma_start(out=mask_sb, in_=mask_ext[:, :])
        nc.sync.dma_start(out=val_sb, in_=vals_ext.rearrange("(t p) v -> p t v", p=128))
        nc.vector.memset(ones_sb, 1.0)
        nc.vector.memset(onesk_sb, 1.0)
        nc.vector.memset(pihalf, PI / 2)
        nc.vector.memset(zero_b, 0.0)
        nc.vector.memset(shift_sb, -5.0)
        make_identity(nc, ident)
        # trigger the Sin table load at t~0 (hidden under DMA/projections)
        nc.scalar.activation(dummy, pihalf, AF.Sin, scale=0.1)

        def ts_abs(out, in_, s1, s2, op0, op1, patch):
            if op1 is None:
                i = nc.vector.tensor_scalar(out, in_, s1, s2, op0=op0)
            else:
                i = nc.vector.tensor_scalar(out, in_, s1, s2, op0=op0, op1=op1)
            abs_patch.append((i.ins.name, patch))
            return i

        scores_a = scores_ps.tile([128, K // 2], F32, tag="sca")
        scores_b = scores_ps.tile([128, K // 2], F32, tag="scb")
        scores_c = [scores_a, scores_b]

        setup_ctx = ExitStack()
        qf_ps_pool = setup_ctx.enter_context(
            tc.tile_pool(name="qf_ps", bufs=1, space="PSUM"))
        kf_ps_pool = setup_ctx.enter_context(
            tc.tile_pool(name="kf_ps", bufs=1, space="PSUM"))

        # ---- projections ----
        ps_q0 = qf_ps_pool.tile([128, Q], F32, tag="psq0")
        ps_q1 = qf_ps_pool.tile([128, Q], F32, tag="psq1")
        for ht, psq in ((0, ps_q0), (1, ps_q1)):
            hsl = slice(ht * 128, (ht + 1) * 128)
            nc.tensor.matmul(psq, wqT_sb[:, 0, hsl], qT_sb[:, 0, :],
                             start=True, stop=False)
            nc.tensor.matmul(psq, wqT_sb[:, 1, hsl], qT_sb[:, 1, :],
                             start=False, stop=True)
            nc.scalar.copy(qf_sb[:, ht, :], psq)

        kf0 = kf_ps_pool.tile([128, K], F32, tag="kf0")
        kf1 = kf_ps_pool.tile([128, K], F32, tag="kf1")
        for ht, kfp in ((0, kf0), (1, kf1)):
            hsl = slice(ht * 128, (ht + 1) * 128)
            for c in range(2):
                csl = slice(c * 512, (c + 1) * 512)
                nc.tensor.matmul(kfp[:, csl], wkT_sb[:, 0, hsl],
                                 kT_sb[:, 0, csl], start=True, stop=False)
                nc.tensor.matmul(kfp[:, csl], wkT_sb[:, 1, hsl],
                                 kT_sb[:, 1, csl], start=False, stop=True)
            nc.scalar.copy(kf_sb[:, ht, :], kfp)
        setup_ctx.close()

        # mask rank-1 (start=True) - first writers of the score banks
        for c in range(2):
            csl = slice(c * 512, (c + 1) * 512)
            nc.tensor.matmul(scores_c[c], ones_sb, mask_sb[:, csl],
                             start=True, stop=False)

        ts_abs(zk_sb, kf_sb, 0.0, None, op0=OP.max, op1=None, patch="op0")

        def emit_kfeat(i):
            w = WB[i]
            nf = _nfolds(w)
            F = f_pool.tile([128, 2, 2, K], F16, tag="F")
            if nf == 0:
                nc.scalar.activation(F[:, 0, :, :], kf_sb, AF.Sin,
                                     bias=zero_b, scale=w)
                nc.scalar.activation(F[:, 1, :, :], kf_sb, AF.Sin,
                                     bias=pihalf, scale=w)
                return F
            C = [4 * PI, 2 * PI, PI][3 - nf:]
            s_w = PI / (2 * w)
            a1 = argk_pool.tile([128, 2, 2, K], F16, tag="ka")
            a2 = argk_pool.tile([128, 2, 2, K], F16, tag="kb")
            ts_abs(a1[:, 0, :, :], kf_sb, s_w, 0.0,
                   op0=OP.subtract, op1=OP.max, patch="op1")
            nc.vector.tensor_scalar(a2[:, 0, :, :], a1[:, 0, :, :], w, C[0],
                                    op0=OP.mult, op1=OP.subtract)
            nc.vector.tensor_scalar(a2[:, 1, :, :], zk_sb, w, C[0],
                                    op0=OP.mult, op1=OP.subtract)
            cur, other = a2, a1
            for c in list(C[1:]) + [PI / 2]:
                ts_abs(other, cur, 0.0, c, op0=OP.max, op1=OP.subtract,
                       patch="op0")
                cur, other = other, cur
            nc.scalar.activation(F, cur, AF.Sin, bias=zero_b, scale=1.0)
            return F

        def emit_qside_r0():
            # j=0 is fold-free: emit its acts + G right after the qf copies
            # so the first score matmuls (chunk j0) release the PE early
            w = WB[0]
            nc.scalar.activation(sincos_q[:, 0, 0, :, :], qf_sb,
                                 AF.Sin, bias=zero_b, scale=w)
            nc.scalar.activation(sincos_q[:, 0, 1, :, :], qf_sb,
                                 AF.Sin, bias=pihalf, scale=w)
            nc.vector.tensor_mul(g_sb[:, 0, :, :, :],
                                 sincos_q[:, 0, :, :, :],
                                 m_sb[:, 0, :, :, :])

        def emit_qside():
            W_EFF = WB + [2 * w for w in WB[2:]]
            ts_abs(zq_sb, qf_sb, 0.0, None, op0=OP.max, op1=None, patch="op0")
            for j, w in enumerate(W_EFF):
                nf = _nfolds(w)
                if nf == 0:
                    nc.scalar.activation(sincos_q[:, j, 0, :, :], qf_sb,
                                         AF.Sin, bias=zero_b, scale=w)
                    nc.scalar.activation(sincos_q[:, j, 1, :, :], qf_sb,
                                         AF.Sin, bias=pihalf, scale=w)
                    continue
                C = [4 * PI, 2 * PI, PI][3 - nf:]
                s_w = PI / (2 * w)
                ts_abs(argq_a[:, j, 0, :, :], qf_sb, s_w, 0.0,
                       op0=OP.subtract, op1=OP.max, patch="op1")
                nc.vector.tensor_scalar(argq_b[:, j, 0, :, :],
                                        argq_a[:, j, 0, :, :], w, C[0],
                                        op0=OP.mult, op1=OP.subtract)
                nc.vector.tensor_scalar(argq_b[:, j, 1, :, :], zq_sb, w, C[0],
                                        op0=OP.mult, op1=OP.subtract)
                L = list(C[1:]) + [PI / 2]
                seq = [argq_b, argq_a, argq_b]
                for ii, c in enumerate(L):
                    dst = argq_f if ii == len(L) - 1 else seq[ii + 1]
                    ts_abs(dst[:, j, :, :, :], seq[ii][:, j, :, :, :], 0.0, c,
                           op0=OP.max, op1=OP.subtract, patch="op0")
            nc.scalar.activation(sincos_q[:, 1:NEFF, :, :, :],
                                 argq_f[:, 1:NEFF, :, :, :],
                                 AF.Sin, bias=zero_b, scale=1.0)
            # G stationaries: per-j TTs (j=0 depends only on the early
            # r0 acts, releasing the first score matmuls ~10us sooner than
            # one full-tile TT would).  Double chunks pair the sin-G with
            # Pc and the cos-G with Ps (F side flipped in the chunk table)
            # so no swapped sincos reads are needed.
            for j in range(NEFF):
                nc.vector.tensor_mul(g_sb[:, j, :, :, :],
                                     sincos_q[:, j, :, :, :],
                                     m_sb[:, j, :, :, :])
            # ones-chunk stationary: w_v * sum_j c_j sin(2 w_j q)
            nc.vector.tensor_scalar_mul(u1_sb, sincos_q[:, 5, 0, :, :],
                                        float(C_DBL[0][2]))
            nc.vector.scalar_tensor_tensor(u2_sb, sincos_q[:, 6, 0, :, :],
                                           float(C_DBL[1][2]), u1_sb,
                                           op0=OP.mult, op1=OP.add)
            nc.vector.scalar_tensor_tensor(u3_sb, sincos_q[:, 7, 0, :, :],
                                           float(C_DBL[2][2]), u2_sb,
                                           op0=OP.mult, op1=OP.add)
            nc.vector.tensor_mul(gone_sb, u3_sb, mw_sb)

        # ---- feature production (k-side) ----
        F_t = {}
        F_t[0] = emit_kfeat(0)
        F_t[1] = emit_kfeat(1)
        F_t[2] = emit_kfeat(2)
        emit_qside()
        P_t = {}
        for idx, i in enumerate((2, 3, 4)):
            if i not in F_t:
                F_t[i] = emit_kfeat(i)
            P = p_pool.tile([128, 2, 2, K], F16, tag="P")
            # Ps = s*c (DVE);  Pc = s^2 (ScalarE Square)
            nc.vector.tensor_mul(P[:, 0, :, :], F_t[i][:, 0, :, :],
                                 F_t[i][:, 1, :, :])
            nc.scalar.activation(P[:, 1, :, :], F_t[i][:, 0, :, :], AF.Square,
                                 bias=zero_b, scale=1.0)
            P_t[5 + idx] = P

        # ---- score matmuls, all deferred (keeps the PE p-state ramped) ----
        # chunk list: (stationary slice provider, moving tile, moving side)
        chunks = []
        for i in range(5):
            chunks.append((("g", i, 0), F_t[i], 1))
            chunks.append((("g", i, 1), F_t[i], 0))
        for j in (5, 6, 7):
            # g[:,j,0] = M0*sin(2w q) pairs Pc (side 1); g[:,j,1] = M1*cos
            # pairs Ps (side 0)
            chunks.append((("g", j, 0), P_t[j], 1))
            chunks.append((("g", j, 1), P_t[j], 0))

        def stat(spec, t):
            kind, j, gs = spec
            return g_sb[:, j, gs, t, :]

        for spec, Fm, fs in chunks:
            for t in range(2):
                for c in range(2):
                    csl = slice(c * 512, (c + 1) * 512)
                    nc.tensor.matmul(scores_c[c], stat(spec, t),
                                     Fm[:, fs, t, csl], start=False, stop=False)
        # ones chunk last: bank-A pair first, with per-bank stop flags
        for c in range(2):
            for t in range(2):
                nc.tensor.matmul(scores_c[c], gone_sb[:, t, :], onesk_sb,
                                 start=False, stop=(t == 1))

        # ---- masked softmax: exp(s-5) with accumulated row sums ----
        for qtr in range(4):
            sc = scores_c[qtr // 2]
            off = (qtr % 2) * 256
            nc.scalar.activation(E_q[qtr], sc[:, off:off + 256], AF.Exp,
                                 bias=shift_sb, scale=1.0, accum_out=rs_q[qtr])
        nc.vector.tensor_add(rowsum, rs_q[0], rs_q[1])
        nc.vector.tensor_add(rs_t, rs_q[2], rs_q[3])
        nc.vector.tensor_add(rowsum, rowsum, rs_t)
        nc.vector.reciprocal(rinv, rowsum)

        # ---- attn @ values ----
        with ExitStack() as tail_ctx:
            tp_ps = tail_ctx.enter_context(
                tc.tile_pool(name="tp_ps", bufs=2, space="PSUM"))
            av_ps = tail_ctx.enter_context(
                tc.tile_pool(name="av_ps", bufs=1, space="PSUM"))
            for kt in range(8):
                E_src = E_q[kt // 2]
                off = (kt % 2) * 128
                tp = tp_ps.tile([128, 128], F16, tag="tp")
                nc.tensor.transpose(tp, E_src[:, off:off + 128], ident)
                nc.vector.tensor_copy(ET_sb[:, kt, :], tp)
            ps_av = av_ps.tile([Q, D], F32)
            for kt in range(8):
                nc.tensor.matmul(ps_av, ET_sb[:, kt, :], val_sb[:, kt, :],
                                 start=(kt == 0), stop=(kt == 7))
            nc.vector.tensor_scalar_mul(out_sb[:, 0:128], ps_av[:, 0:128], rinv)
            nc.sync.dma_start(out=out_ext[:, 0:128], in_=out_sb[:, 0:128])
            nc.vector.tensor_scalar_mul(out_sb[:, 128:256], ps_av[:, 128:256], rinv)
        nc.sync.dma_start(out=out_ext[:, 128:256], in_=out_sb[:, 128:256])

    _patch_bir(nc, abs_patch)
    return nc


def _patch_bir(nc, abs_patch):
    """Rewrite recorded max-placeholders to the walrus "abs" ALU op, and
    fix multi-wait instructions (walrus accepts one sync wait each): drop
    redundant same-engine waits on in-order compute queues, hoist the rest
    onto single-wait EventSemaphore carriers."""
    import json

    d = json.loads(nc.to_json_bytes())
    patch_map = dict(abs_patch)
    k = [0]
    self_drop = {"Activation": "Activation", "DVE": "DVE"}
    compute_ops = {"Activation", "TensorScalarPtr", "TensorScalar", "TensorTensor",
                   "TensorCopy", "TensorReduce", "Reciprocal", "Memset"}
    n_abs = 0
    for fn in d["functions"]:
        for blk in fn["blocks"]:
            out = []
            for inst in blk["instructions"]:
                slot = patch_map.get(inst.get("name"))
                if slot is not None:
                    inst[slot] = "abs"
                    n_abs += 1
                si = inst.get("sync_info") or {}
                ow = si.get("on_wait") or []
                op = inst.get("opcode")
                eng = inst.get("engine")
                if len(ow) > 1 and op != "EventSemaphore":
                    if op in compute_ops and eng in self_drop:
                        pref = self_drop[eng] + "_"
                        ow = [w for w in ow
                              if not str(w.get("ant_name", "")).startswith(pref)]
                    while len(ow) > 1:
                        w = ow.pop(0)
                        k[0] += 1
                        out.append({
                            "debug": inst.get("debug", 0), "engine": eng,
                            "ins": [], "name": f"WSplit-{k[0]}",
                            "opcode": "EventSemaphore", "outs": [],
                            "sync_info": {"on_update": [], "on_wait": [w]},
                        })
                    si["on_wait"] = ow
                out.append(inst)
            blk["instructions"] = out
    assert n_abs == len(abs_patch), (n_abs, len(abs_patch))
    patched = json.dumps(d).encode()
    nc.to_json_bytes = lambda: patched


def _get_nc():
    if "nc" not in _CACHE:
        _CACHE["nc"] = _build_bass()
    return _CACHE["nc"]


def _host_prep(queries, keys, values, W_q, W_k, w_v, valid_lens):
    """Build the 8 per-core input maps."""
    queries = np.asarray(queries, dtype=np.float32)
    keys = np.asarray(keys, dtype=np.float32)
    values = np.asarray(values, dtype=np.float32)
    W_q = np.asarray(W_q, dtype=np.float32)
    W_k = np.asarray(W_k, dtype=np.float32)
    w_v = np.asarray(w_v, dtype=np.float32)
    valid = np.asarray(valid_lens).astype(np.int64)

    wqT = np.ascontiguousarray(W_q.T.astype(np.float16))
    wkT = np.ascontiguousarray(W_k.T.astype(np.float16))
    # M[p, j, side, t, q] = w_v[t*128 + p] * coef[j, side]
    wv2 = w_v.reshape(2, 128).T                              # (p, t)
    coef = np.zeros((NEFF, 2), np.float32)
    for i, (c0, c1) in enumerate(C_BASE):
        coef[i] = (c0, c1)
    for idx, (cps, cpc, _) in enumerate(C_DBL):
        coef[5 + idx] = (cpc, cps)      # side0 = sin-G (pairs Pc), side1 = cos-G
    M = np.ascontiguousarray(
        (wv2[:, None, None, :, None] * coef[None, :, :, None, None]
         ).astype(np.float16) * np.ones((1, 1, 1, 1, Q), np.float16))
    Mw = np.ascontiguousarray(
        np.broadcast_to(wv2[:, :, None], (128, 2, Q)).astype(np.float16))

    kidx = np.arange(K)
    in_maps = []
    for b in range(B):
        mask = np.where(kidx < valid[b], np.float16(0.0), np.float16(NEG_BIG))
        in_maps.append({
            "qT": np.ascontiguousarray(queries[b].T.astype(np.float16)),
            "kT": np.ascontiguousarray(keys[b].T.astype(np.float16)),
            "vals": np.ascontiguousarray(values[b].astype(np.float16)),
            "wqT": wqT,
            "wkT": wkT,
            "M": M,
            "Mw": Mw,
            "mask": np.ascontiguousarray(mask.reshape(1, K)),
        })
    return in_maps, valid, values


def _run(inputs, trace=False, **kw):
    from concourse.bass_utils import run_bass_kernel_spmd

    nc = _get_nc()
    in_maps, valid, values = _host_prep(**inputs)
    res = run_bass_kernel_spmd(nc, in_maps, list(range(B)), trace=trace, **kw)
    out = np.stack([np.asarray(res.results[i]["out"], dtype=np.float32)
                    for i in range(B)])
    for b in range(B):
        if valid[b] == 0:
            out[b] = np.broadcast_to(values[b].mean(axis=0), (Q, D))
    return out, res


def kernel(**inputs):
    out, _ = _run(inputs, trace=False)
    return out
